# revision 11
# baseline (speedup 1.0000x reference)
"""Full on-device Trainium2 kernel for the DNA entropy transformer.

Sharding: sequence-parallel over the flattened (B*L) token stream, 8 cores x
4096 tokens, each with a 4-block (2048-token) left halo that is recomputed
locally (receptive field of the 512-wide sliding-window attention is 1 block
per layer, 4 layers) -- so there is no inter-core communication at all.

On-device layout:
  - activations feature-partition: x/h/Q^T/K^T as [2*128, 6144] bf16
  - V token-partition [tok, head*65] with a fused ones-column so the PV matmul
    produces both the (unnormalized) attention output and the softmax
    denominator in one accumulation group
  - attention is organized per key-block: one key block attends against the
    1024 queries of (its own block (causal) + the next block (prev-window)),
    the band/ALiBi mask is applied multiplicatively after exp() via a single
    resident exp(bias) tile per head (pattern is shift-invariant)
  - LayerNorm gains/biases, the attention scale, and the final LN are folded
    into the weight matrices on the host; LN on device is a pure standardize
    whose per-token A=1/sigma, B=-mu/sigma rows are broadcast across
    partitions with rank-1 matmuls.
"""
import sys
sys.path.insert(0, "/opt/trn_rl_repo")
import numpy as np
import ml_dtypes
import jax

try:
    jax.config.update("jax_compilation_cache_dir", "/root/.jax_bass_cache")
    jax.config.update("jax_persistent_cache_min_entry_size_bytes", 0)
    jax.config.update("jax_persistent_cache_min_compile_time_secs", 0.0)
except Exception:
    pass

import concourse.bass as bass
import concourse.tile as tile
from concourse import mybir
from concourse.bass_utils import run_bass_kernel_spmd

BF16_NP = ml_dtypes.bfloat16
F32 = mybir.dt.float32
F16 = mybir.dt.float16
BF16 = mybir.dt.bfloat16
AF = mybir.ActivationFunctionType
OP = mybir.AluOpType

B, L, D, H, NL, FF, W, V = 4, 8192, 256, 4, 4, 1024, 512, 14
DH = D // H
P = 128
N_CORES = 8
RB = 8            # real blocks per core
HALO = 4          # halo blocks (= NL)
NB = RB + HALO    # 12 local blocks
T = NB * W        # 6144 local tokens
TR = RB * W       # 4096 real tokens per core
DC = D // P       # 2 feature chunks

_NC = None


def _split_syncs(nc, max_waits=1, max_updates=2):
    dummy = nc.alloc_semaphore("wsplit_dummy")
    for fn in nc.m.functions:
        for blk in fn.blocks:
            out = []
            for ins in blk.instructions:
                si = ins.sync_info
                if si is None:
                    out.append(ins)
                    continue
                waits = list(si.on_wait or [])
                updates = list(si.on_update or [])
                pre = []
                while len(waits) > max_waits:
                    chunk, waits = waits[:max_waits], waits[max_waits:]
                    pre.append(mybir.InstEventSemaphore(
                        name=f"{ins.name}-ws{len(pre)}", engine=ins.engine,
                        sync_info=mybir.SyncInfo(on_wait=chunk, on_update=[
                            mybir.SyncUpdate(sync_type="semaphore", id=dummy.num,
                                             update_mode="sem-inc", update_value=1,
                                             ant_name="wsplit_dummy")])))
                post = []
                if "DMA" not in type(ins).__name__:
                    while len(updates) > max_updates:
                        chunk = updates[-max_updates:]
                        updates = updates[:-max_updates]
                        post.append(mybir.InstEventSemaphore(
                            name=f"{ins.name}-us{len(post)}", engine=ins.engine,
                            sync_info=mybir.SyncInfo(on_wait=[], on_update=chunk)))
                ins.sync_info = mybir.SyncInfo(on_wait=waits, on_update=updates)
                out.extend(pre)
                out.append(ins)
                out.extend(post)
            blk.instructions = out
    return nc


def _bcast_ap(ap, parts):
    """DRAM AP whose leading (partition) dim is a broadcast of `parts` copies."""
    return bass.AP(tensor=ap.tensor, offset=ap.offset,
                   ap=[[0, parts]] + [list(p) for p in ap.ap[1:]])


def _build():
    global _NC
    if _NC is not None:
        return _NC
    nc = bass.Bass()

    wpack_d = nc.declare_dram_parameter("wpack", [P, NL, 6144], BF16, isOutput=False)
    bcols_d = nc.declare_dram_parameter("bcols", [P, 16 * NL], F32, isOutput=False)
    bvrow_d = nc.declare_dram_parameter("bvrow", [NL, D], F32, isOutput=False)
    emb_d = nc.declare_dram_parameter("emb", [V, D], BF16, isOutput=False)
    viota_d = nc.declare_dram_parameter("viota", [V, 1], F32, isOutput=False)
    expb_d = nc.declare_dram_parameter("expb", [P, H * 640], BF16, isOutput=False)
    hw_d = nc.declare_dram_parameter("hw", [P, DC * V], BF16, isOutput=False)
    hb_d = nc.declare_dram_parameter("hb", [V, 1], F32, isOutput=False)
    ids_d = nc.declare_dram_parameter("ids", [1, T], F32, isOutput=False)
    fcol_d = nc.declare_dram_parameter("fcol", [1, NB], F32, isOutput=False)
    lgT_d = nc.declare_dram_parameter("logitsT", [V, TR], F16, isOutput=True)

    with tile.TileContext(nc) as tc:
        with tc.tile_pool(name="const", bufs=1) as constp, \
             tc.tile_pool(name="persist", bufs=1) as pers, \
             tc.tile_pool(name="wpool", bufs=2) as wpool, \
             tc.tile_pool(name="bvp", bufs=2) as bvp, \
             tc.tile_pool(name="gelu", bufs=10) as gpool, \
             tc.tile_pool(name="ptile", bufs=3) as ptp, \
             tc.tile_pool(name="etile", bufs=2) as etp, \
             tc.tile_pool(name="work", bufs=2) as work, \
             tc.tile_pool(name="rowsf", bufs=2) as rowsf, \
             tc.tile_pool(name="rowsb", bufs=2) as rowsb, \
             tc.tile_pool(name="ltp", bufs=2) as ltp, \
             tc.tile_pool(name="psA", bufs=2, space="PSUM") as psA, \
             tc.tile_pool(name="psO", bufs=3, space="PSUM") as psO, \
             tc.tile_pool(name="psZ", bufs=1, space="PSUM") as psZ:

            # ---- constants
            emb_sb = constp.tile([V, D], BF16)
            nc.sync.dma_start(out=emb_sb, in_=emb_d[:, :])
            viota_sb = constp.tile([V, 1], F32)
            nc.sync.dma_start(out=viota_sb, in_=viota_d[:, :])
            expb_sb = constp.tile([P, H * 640], BF16)
            nc.sync.dma_start(out=expb_sb, in_=expb_d[:, :])
            bcols_sb = constp.tile([P, 16 * NL], F32)
            nc.sync.dma_start(out=bcols_sb, in_=bcols_d[:, :])
            hw_sb = constp.tile([P, DC * V], BF16)
            nc.sync.dma_start(out=hw_sb, in_=hw_d[:, :])
            hb_sb = constp.tile([V, 1], F32)
            nc.sync.dma_start(out=hb_sb, in_=hb_d[:, :])
            fsb = constp.tile([P, NB], F32)
            nc.gpsimd.dma_start(out=fsb, in_=_bcast_ap(fcol_d[0:1, :], P))
            ones1 = constp.tile([1, P], BF16)
            nc.vector.memset(ones1, 1.0)
            oneD = constp.tile([P, 1], BF16)
            nc.vector.memset(oneD, 1.0 / D)
            eps_sb = constp.tile([1, 1], F32)
            nc.vector.memset(eps_sb, 1e-5)

            # ---- persistent activation tensors (feature-partition)
            x_t = pers.tile([P, DC, T], BF16, tag="x")
            qt_t = pers.tile([P, DC, T], BF16, tag="qt")
            kt_t = pers.tile([P, DC, T], BF16, tag="kt")
            hot_t = pers.tile([P, DC, T], BF16, tag="hot")  # h / attn-out / h2
            vp_t = pers.tile([P, NB * 4 * H * 65], BF16, tag="vp")  # [tok-tile, h, 65]
            for vi in range(NB * 4):
                sl = vp_t[:, vi * 260:(vi + 1) * 260].rearrange(
                    "p (h c) -> p h c", c=65)
                nc.vector.memset(sl[:, :, 64:65], 1.0)

            # ---- embedding: one-hot matmul per 512-token slice
            for ts in range(NB):
                idsb = work.tile([V, W], BF16, tag="ids")
                nc.gpsimd.dma_start(
                    out=idsb, in_=_bcast_ap(ids_d[0:1, ts * W:(ts + 1) * W], V))
                oh = work.tile([V, W], BF16, tag="oh")
                nc.vector.tensor_scalar(oh, idsb, viota_sb, None, OP.is_equal)
                for dc in range(DC):
                    ps = psA.tile([P, W], F32, tag="big")
                    nc.tensor.matmul(ps, emb_sb[:, dc * P:(dc + 1) * P], oh,
                                     start=True, stop=True)
                    nc.vector.tensor_copy(x_t[:, dc, ts * W:(ts + 1) * W], ps)

            def layernorm(ts_range, dest_t):
                for ts in ts_range:
                    c0, c1 = ts * W, (ts + 1) * W
                    stpm = psO.tile([1, W], F32, tag="ot", name="stpm")
                    for dc in range(DC):
                        nc.tensor.matmul(stpm, oneD, x_t[:, dc, c0:c1],
                                         start=(dc == 0), stop=(dc == DC - 1))
                    stps = psO.tile([1, W], F32, tag="ot", name="stps")
                    for dc in range(DC):
                        sq = work.tile([P, W], BF16, tag="sq")
                        nc.vector.tensor_mul(sq, x_t[:, dc, c0:c1], x_t[:, dc, c0:c1])
                        nc.tensor.matmul(stps, oneD, sq,
                                         start=(dc == 0), stop=(dc == DC - 1))
                    mrow = rowsf.tile([1, W], F32, tag="mrow")
                    nc.vector.tensor_copy(mrow, stpm)
                    msq = rowsf.tile([1, W], F32, tag="msq")
                    nc.vector.tensor_mul(msq, mrow, mrow)
                    varr = rowsf.tile([1, W], F32, tag="var")
                    nc.vector.scalar_tensor_tensor(
                        varr, stps, 0.0, msq, OP.bypass, OP.subtract)
                    stdr = rowsf.tile([1, W], F32, tag="std")
                    nc.scalar.activation(stdr, varr, AF.Sqrt, bias=eps_sb)
                    ar = rowsb.tile([1, W], BF16, tag="ar")
                    with nc.allow_low_precision(reason="bf16 LN scale"):
                        nc.vector.reciprocal(ar, stdr)
                    br = rowsb.tile([1, W], BF16, tag="br")
                    nc.vector.scalar_tensor_tensor(
                        br, mrow, -1.0, ar, OP.mult, OP.mult)
                    Ab = psA.tile([P, W], F32, tag="big")
                    nc.tensor.matmul(Ab, ones1, ar, start=True, stop=True)
                    Bb = psA.tile([P, W], F32, tag="big")
                    nc.tensor.matmul(Bb, ones1, br, start=True, stop=True)
                    for dc in range(DC):
                        t1 = work.tile([P, W], BF16, tag="t1")
                        nc.vector.scalar_tensor_tensor(
                            t1, Ab, 0.0, x_t[:, dc, c0:c1], OP.bypass, OP.mult)
                        nc.vector.scalar_tensor_tensor(
                            dest_t[:, dc, c0:c1], Bb, 0.0, t1, OP.bypass, OP.add)

            # ================= layers =================
            for l in range(NL):
                l0 = l
                wl = wpool.tile([P, 6144], BF16, tag="wl")
                nc.sync.dma_start(out=wl, in_=wpack_d[:, l, :])
                bvb = bvp.tile([P, D], BF16, tag="bvb")
                nc.gpsimd.dma_start(out=bvb, in_=_bcast_ap(bvrow_d[l:l + 1, :], P))

                # ---- LN1 -> hot
                layernorm(range(l0, NB), hot_t)

                # ---- Q (blocks l0+1..), K (blocks l0..) projections
                for p3, dst, trange in ((0, qt_t, range(l0 + 1, NB)),
                                        (1, kt_t, range(l0, NB))):
                    for oc in range(DC):
                        for ts in trange:
                            ps = psA.tile([P, W], F32, tag="big")
                            for dc in range(DC):
                                tw = p3 * 4 + dc * 2 + oc
                                nc.tensor.matmul(
                                    ps, wl[:, tw * P:(tw + 1) * P],
                                    hot_t[:, dc, ts * W:(ts + 1) * W],
                                    start=(dc == 0), stop=(dc == DC - 1))
                            nc.vector.tensor_scalar(
                                dst[:, oc, ts * W:(ts + 1) * W], ps,
                                bcols_sb[:, l * 16 + p3 * 2 + oc:l * 16 + p3 * 2 + oc + 1],
                                None, OP.add)

                # ---- V projection (token-partition, per 128-token tile)
                for vi in range(4 * l0, 4 * NB):
                    ps = psA.tile([P, D], F32, tag="big")
                    for dc in range(DC):
                        nc.tensor.matmul(
                            ps, hot_t[:, dc, vi * P:(vi + 1) * P],
                            wl[:, 1536 + dc * D:1536 + (dc + 1) * D],
                            start=(dc == 0), stop=(dc == DC - 1))
                    dst = vp_t[:, vi * 260:(vi + 1) * 260].rearrange(
                        "p (h c) -> p h c", c=65)[:, :, 0:64]
                    nc.vector.scalar_tensor_tensor(
                        dst, ps.rearrange("p (h c) -> p h c", c=64), 0.0,
                        bvb.rearrange("p (h c) -> p h c", c=64),
                        OP.bypass, OP.add)

                # ---- attention, merged per key-block
                for h_ in range(H):
                    ch = h_ // 2
                    r0 = (h_ % 2) * 64
                    ot = {}
                    for n in range(l0, NB):
                        for kc in range(4):
                            kb = n * W + kc * P
                            b0 = kc * P          # piece A at tile cols [b0, W)
                            wB = kc * P + P      # piece B at tile cols [W, W+wB)
                            a0 = b0 if n > l0 else W
                            a1 = (W + wB) if n < NB - 1 else W
                            st = psA.tile([P, 1024], F32, tag="big")
                            lhsT = kt_t[r0:r0 + 64, ch, kb:kb + P]
                            if n > l0:
                                nc.tensor.matmul(
                                    st[:, b0:W], lhsT,
                                    qt_t[r0:r0 + 64, ch, kb:n * W + W],
                                    start=True, stop=True)
                            if n < NB - 1:
                                nc.tensor.matmul(
                                    st[:, W:W + wB], lhsT,
                                    qt_t[r0:r0 + 64, ch,
                                         (n + 1) * W:(n + 1) * W + wB],
                                    start=True, stop=True)
                            ep = etp.tile([P, 1024], BF16, tag="ep")
                            nc.scalar.activation(ep[:, a0:a1], st[:, a0:a1], AF.Exp)
                            pt = ptp.tile([P, 1024], BF16, tag="pt")
                            nc.vector.tensor_mul(
                                pt[:, a0:a1], ep[:, a0:a1],
                                expb_sb[:, h_ * 640 + a0 - b0:h_ * 640 + a1 - b0])
                            if n == 3 and n < NB - 1:
                                nc.vector.tensor_scalar_mul(
                                    pt[:, W:W + wB], pt[:, W:W + wB], fsb[:, 3:4])
                            vT = vp_t[:, (4 * n + kc) * 260 + h_ * 65:
                                      (4 * n + kc) * 260 + (h_ + 1) * 65]
                            if n > l0:
                                first = n not in ot
                                if first:
                                    ot[n] = psO.tile([65, W], F32, tag="ot", name=f"ot{n}")
                                nc.tensor.matmul(
                                    ot[n][:, b0:W], vT, pt[:, b0:W],
                                    start=first, stop=(kc == 3),
                                    skip_group_check=True)
                            if n < NB - 1:
                                first = (n + 1) not in ot
                                if first:
                                    ot[n + 1] = psO.tile([65, W], F32, tag="ot", name=f"ot{n+1}")
                                nc.tensor.matmul(
                                    ot[n + 1][:, 0:wB], vT, pt[:, W:W + wB],
                                    start=first, stop=False,
                                    skip_group_check=True)
                        # normalize q-block n once its last contribution landed
                        if n > l0:
                            rz = rowsb.tile([1, W], BF16, tag="rz")
                            with nc.allow_low_precision(reason="bf16 softmax z"):
                                nc.vector.reciprocal(rz, ot[n][64:65, :])
                            zb = psZ.tile([64, W], F32, tag="zb")
                            nc.tensor.matmul(zb, ones1[0:1, 0:64], rz,
                                             start=True, stop=True)
                            zbs = rowsb.tile([64, W], BF16, tag="zbs")
                            nc.vector.tensor_copy(zbs, zb)
                            nc.vector.tensor_mul(
                                hot_t[r0:r0 + 64, ch, n * W:(n + 1) * W],
                                ot[n][0:64, :], zbs)
                            del ot[n]

                # ---- O projection + residual
                for oc in range(DC):
                    for ts in range(l0 + 1, NB):
                        ps = psA.tile([P, W], F32, tag="big")
                        for dc in range(DC):
                            tw = 8 + dc * 2 + oc
                            nc.tensor.matmul(
                                ps, wl[:, tw * P:(tw + 1) * P],
                                hot_t[:, dc, ts * W:(ts + 1) * W],
                                start=(dc == 0), stop=(dc == DC - 1))
                        nc.vector.scalar_tensor_tensor(
                            x_t[:, oc, ts * W:(ts + 1) * W], ps,
                            bcols_sb[:, l * 16 + 4 + oc:l * 16 + 4 + oc + 1],
                            x_t[:, oc, ts * W:(ts + 1) * W], OP.add, OP.add)

                # ---- LN2 -> hot
                layernorm(range(l0 + 1, NB), hot_t)

                # ---- FFN
                for ts in range(l0 + 1, NB):
                    gts = []
                    for oc in range(8):
                        ps = psA.tile([P, W], F32, tag="big")
                        for dc in range(DC):
                            tw = dc * 8 + oc
                            nc.tensor.matmul(
                                ps, wl[:, 2048 + tw * P:2048 + (tw + 1) * P],
                                hot_t[:, dc, ts * W:(ts + 1) * W],
                                start=(dc == 0), stop=(dc == DC - 1))
                        gt = gpool.tile([P, W], BF16, tag="gt")
                        nc.scalar.activation(
                            gt, ps, AF.Gelu_apprx_tanh,
                            bias=bcols_sb[:, l * 16 + 8 + oc:l * 16 + 8 + oc + 1])
                        gts.append(gt)
                    for oc2 in range(DC):
                        ps2 = psO.tile([P, W], F32, tag="ot")
                        for dc in range(8):
                            tw = dc * 2 + oc2
                            nc.tensor.matmul(
                                ps2, wl[:, 4096 + tw * P:4096 + (tw + 1) * P],
                                gts[dc], start=(dc == 0), stop=(dc == 7))
                        nc.vector.scalar_tensor_tensor(
                            x_t[:, oc2, ts * W:(ts + 1) * W], ps2,
                            bcols_sb[:, l * 16 + 6 + oc2:l * 16 + 6 + oc2 + 1],
                            x_t[:, oc2, ts * W:(ts + 1) * W], OP.add, OP.add)

            # ---- final LN + head
            layernorm(range(HALO, NB), hot_t)
            for ts in range(HALO, NB):
                ps = psA.tile([V, W], F32, tag="big")
                for dc in range(DC):
                    nc.tensor.matmul(ps, hw_sb[:, dc * V:(dc + 1) * V],
                                     hot_t[:, dc, ts * W:(ts + 1) * W],
                                     start=(dc == 0), stop=(dc == DC - 1))
                lt = ltp.tile([V, W], F16, tag="lt")
                with nc.allow_low_precision(reason="f16 logits output"):
                    nc.vector.tensor_scalar(lt, ps, hb_sb, None, OP.add)
                nc.sync.dma_start(
                    out=lgT_d[:, (ts - HALO) * W:(ts - HALO + 1) * W], in_=lt)

    _split_syncs(nc)
    _NC = nc
    return nc


def nA2(kc):
    return kc * P + P


def _prep_shared(inputs):
    """Pack the (input-independent) weight tensors. Returns dict of np arrays."""
    f32 = np.float32
    emb = inputs["embed"].astype(f32)
    wq, bq = inputs["wq"].astype(f32), inputs["bq"].astype(f32)
    wk, bk = inputs["wk"].astype(f32), inputs["bk"].astype(f32)
    wv, bv = inputs["wv"].astype(f32), inputs["bv"].astype(f32)
    wo, bo = inputs["wo"].astype(f32), inputs["bo"].astype(f32)
    g1, c1 = inputs["ln1_g"].astype(f32), inputs["ln1_b"].astype(f32)
    g2, c2 = inputs["ln2_g"].astype(f32), inputs["ln2_b"].astype(f32)
    w1, b1 = inputs["w1"].astype(f32), inputs["b1"].astype(f32)
    w2, b2 = inputs["w2"].astype(f32), inputs["b2"].astype(f32)
    gf, cf = inputs["lnf_g"].astype(f32), inputs["lnf_b"].astype(f32)
    hw = inputs["head_w"].astype(f32)

    scale = f32(1.0 / np.sqrt(DH))
    wpack = np.zeros((P, NL, 6144), BF16_NP)
    bcols = np.zeros((P, 16 * NL), f32)
    bvrow = np.zeros((NL, D), f32)
    for l in range(NL):
        wqe = (g1[l][:, None] * wq[l]) * scale
        bqe = (c1[l] @ wq[l] + bq[l]) * scale
        wke = g1[l][:, None] * wk[l]
        bke = c1[l] @ wk[l] + bk[l]
        wve = g1[l][:, None] * wv[l]
        bve = c1[l] @ wv[l] + bv[l]
        w1e = g2[l][:, None] * w1[l]
        b1e = c2[l] @ w1[l] + b1[l]
        for p3, Wm in ((0, wqe), (1, wke), (2, wo[l])):
            for dc in range(DC):
                for oc in range(DC):
                    t = p3 * 4 + dc * 2 + oc
                    wpack[:, l, t * P:(t + 1) * P] = \
                        Wm[dc * P:(dc + 1) * P, oc * P:(oc + 1) * P]
        for dc in range(DC):
            wpack[:, l, 1536 + dc * D:1536 + (dc + 1) * D] = \
                wve[dc * P:(dc + 1) * P, :]
        for dc in range(DC):
            for oc in range(8):
                t = dc * 8 + oc
                wpack[:, l, 2048 + t * P:2048 + (t + 1) * P] = \
                    w1e[dc * P:(dc + 1) * P, oc * P:(oc + 1) * P]
        for dc in range(8):
            for oc in range(DC):
                t = dc * 2 + oc
                wpack[:, l, 4096 + t * P:4096 + (t + 1) * P] = \
                    w2[l][dc * P:(dc + 1) * P, oc * P:(oc + 1) * P]
        for oc in range(DC):
            bcols[:, l * 16 + 0 + oc] = bqe[oc * P:(oc + 1) * P]
            bcols[:, l * 16 + 2 + oc] = bke[oc * P:(oc + 1) * P]
            bcols[:, l * 16 + 4 + oc] = bo[l][oc * P:(oc + 1) * P]
            bcols[:, l * 16 + 6 + oc] = b2[l][oc * P:(oc + 1) * P]
        for oc in range(8):
            bcols[:, l * 16 + 8 + oc] = b1e[oc * P:(oc + 1) * P]
        bvrow[l] = bve

    hwe = (gf[:, None] * hw)
    hbe = cf @ hw
    hwp = np.zeros((P, DC * V), BF16_NP)
    for dc in range(DC):
        hwp[:, dc * V:(dc + 1) * V] = hwe[dc * P:(dc + 1) * P, :]

    slopes = np.exp2(-8.0 * np.arange(1, H + 1, dtype=f32) / H)
    pp = np.arange(P, dtype=f32)[:, None]
    tt = np.arange(640, dtype=f32)[None, :]
    dist = tt - pp
    expb = np.zeros((P, H * 640), BF16_NP)
    for h_ in range(H):
        eb = np.exp(-slopes[h_] * dist) * ((dist >= 0) & (dist <= W))
        expb[:, h_ * 640:(h_ + 1) * 640] = eb.astype(BF16_NP)

    shared = {
        "wpack": wpack,
        "bcols": bcols,
        "bvrow": bvrow,
        "emb": emb.astype(BF16_NP),
        "viota": np.arange(V, dtype=f32)[:, None],
        "expb": expb,
        "hw": hwp,
        "hb": hbe[:, None].astype(f32),
    }
    return shared


def _fcol_concat():
    fcol = np.ones((N_CORES, NB), np.float32)
    for c in range(N_CORES):
        for n in range(NB):
            g = RB * c + (n + 1) - HALO
            if g % (L // W) == 0:
                fcol[c, n] = 0.0
    return fcol


def _ids_concat(byte_ids):
    """Per-core halo'd token streams, concatenated on axis 0: [N_CORES, T]."""
    flat_ids = np.asarray(byte_ids).reshape(-1)
    NT = flat_ids.shape[0]
    out = np.empty((N_CORES, T), np.float32)
    for c in range(N_CORES):
        idx = (np.arange(c * TR - HALO * W, c * TR + TR)) % NT
        out[c] = flat_ids[idx].astype(np.float32)
    return out


def _prep(inputs):
    """Fallback-path per-core input maps (original contract)."""
    shared = _prep_shared(inputs)
    ids = _ids_concat(inputs["byte_ids"])
    fcol = _fcol_concat()
    in_maps = []
    for c in range(N_CORES):
        m = dict(shared)
        m["ids"] = ids[c:c + 1]
        m["fcol"] = fcol[c:c + 1]
        in_maps.append(m)
    return in_maps


# ---------------------------------------------------------------------------
# Fast dispatch: compile the shard_map executable once, keep the (static)
# packed weights resident on the 8 devices, and per invocation only upload the
# ids stream (197KB), execute, and fetch the logits. The previous invocation's
# output buffer is donated back as the kernel's output-init operand (the
# kernel writes every element of logitsT, so its initial contents are dead).
# ---------------------------------------------------------------------------
_DISP = None


def _make_dispatch(shared):
    import jax.numpy as jnp
    from jax.sharding import Mesh, PartitionSpec, NamedSharding
    from jax.experimental.shard_map import shard_map
    from concourse.bass2jax import (
        _bass_exec_p, partition_id_tensor, install_neuronx_cc_hook)

    nc = _build()
    install_neuronx_cc_hook()

    part_name = nc.partition_id_tensor.name if nc.partition_id_tensor else None
    in_names, out_names, out_avals = [], [], []
    for alloc in nc.m.functions[0].allocations:
        if not isinstance(alloc, mybir.MemoryLocationSet):
            continue
        name = alloc.memorylocations[0].name
        if alloc.kind == "ExternalInput":
            if name != part_name:
                in_names.append(name)
        elif alloc.kind == "ExternalOutput":
            out_names.append(name)
            out_avals.append(jax.core.ShapedArray(
                tuple(alloc.tensor_shape), mybir.dt.np(alloc.dtype)))
    n_params, n_outs = len(in_names), len(out_avals)
    all_in = in_names + out_names + ([part_name] if part_name else [])

    def _body(*args):
        operands = list(args)
        if part_name is not None:
            operands.append(partition_id_tensor())
        return tuple(_bass_exec_p.bind(
            *operands, out_avals=tuple(out_avals), in_names=tuple(all_in),
            out_names=tuple(out_names), lowering_input_output_aliases=(),
            sim_require_finite=True, sim_require_nnan=True, nc=nc))

    devices = jax.devices()[:N_CORES]
    mesh = Mesh(np.asarray(devices), ("core",))
    shard = NamedSharding(mesh, PartitionSpec("core"))
    donate = tuple(range(n_params, n_params + n_outs))
    sharded = jax.jit(
        shard_map(_body, mesh=mesh,
                  in_specs=(PartitionSpec("core"),) * (n_params + n_outs),
                  out_specs=(PartitionSpec("core"),) * n_outs,
                  check_rep=False),
        donate_argnums=donate, keep_unused=True)

    ids_idx = in_names.index("ids")
    fcol = _fcol_concat()
    dev_in = []
    for nm in in_names:
        if nm == "ids":
            arr = np.zeros((N_CORES, T), np.float32)
        elif nm == "fcol":
            arr = fcol
        else:
            a = shared[nm]
            arr = np.concatenate([a] * N_CORES, axis=0)
        dev_in.append(jax.device_put(arr, shard))
    (ov,) = out_avals
    out0 = jax.jit(lambda: jnp.zeros((N_CORES * ov.shape[0],) + ov.shape[1:],
                                     ov.dtype), out_shardings=shard)()
    jax.block_until_ready(dev_in)
    return {"sharded": sharded, "dev_in": dev_in, "ids_idx": ids_idx,
            "shard": shard, "out": out0, "ov": ov, "shared": shared}


def _ensure_dispatch(shared):
    global _DISP
    if _DISP is not None:
        prev = _DISP["shared"]
        if prev is shared or \
                all(np.array_equal(prev[k], shared[k]) for k in shared):
            return _DISP
        _DISP = None                    # weights changed: rebuild residency
    _DISP = _make_dispatch(shared)
    return _DISP


def _invoke(ids_np):
    """One full device invocation: upload ids, execute on 8 cores, fetch and
    assemble the [B, L, V] logits on host."""
    d = _DISP
    ids_dev = jax.device_put(ids_np, d["shard"])
    args = list(d["dev_in"])
    args[d["ids_idx"]] = ids_dev
    out = d["sharded"](*args, d["out"])[0]
    d["out"] = out
    arr = np.asarray(out)                              # [8*V, TR]
    return np.ascontiguousarray(
        arr.reshape(N_CORES, V, TR).transpose(0, 2, 1)).reshape(B, L, V)


_PREP_CACHE = {}


def _weights_digest(inputs):
    import hashlib
    h = hashlib.blake2b(digest_size=16)
    for k in sorted(inputs):
        if k != "byte_ids":
            a = np.ascontiguousarray(inputs[k])
            h.update(k.encode());  h.update(a.tobytes())
    return h.hexdigest()


def kernel(**inputs):
    inputs = {k: np.asarray(v) for k, v in inputs.items()}
    try:
        dig = _weights_digest(inputs)
        if dig in _PREP_CACHE:
            shared = _PREP_CACHE[dig]
        else:
            shared = _prep_shared(inputs)
            _PREP_CACHE.clear()
            _PREP_CACHE[dig] = shared
        _ensure_dispatch(shared)
        ids = _ids_concat(inputs["byte_ids"])
        return _invoke(ids).astype(np.float32)
    except Exception:
        import traceback
        traceback.print_exc()
        # conservative fallback: original run_bass_kernel_spmd path
        in_maps = _prep(inputs)
        nc = _build()
        res = run_bass_kernel_spmd(nc, in_maps, list(range(N_CORES)))
        parts = [res.results[c]["logitsT"].T for c in range(N_CORES)]
        return np.concatenate(parts, axis=0).reshape(B, L, V).astype(np.float32)



# revision 19
# speedup vs baseline: 1.5281x; 1.5281x over previous
"""Full on-device Trainium2 kernel for the DNA entropy transformer.

Sharding: sequence-parallel over the flattened (B*L) token stream, 8 cores x
4096 tokens, each with a 4-block (2048-token) left halo that is recomputed
locally (receptive field of the 512-wide sliding-window attention is 1 block
per layer, 4 layers) -- so there is no inter-core communication at all.

On-device layout:
  - activations feature-partition: x/h/Q^T/K^T as [2*128, 6144] bf16
  - V token-partition [tok, head*65] with a fused ones-column so the PV matmul
    produces both the (unnormalized) attention output and the softmax
    denominator in one accumulation group
  - attention is organized per key-block: one key block attends against the
    1024 queries of (its own block (causal) + the next block (prev-window)),
    the band/ALiBi mask is applied multiplicatively after exp() via a single
    resident exp(bias) tile per head (pattern is shift-invariant)
  - LayerNorm gains/biases, the attention scale, and the final LN are folded
    into the weight matrices on the host; LN on device is a pure standardize
    whose per-token A=1/sigma, B=-mu/sigma rows are broadcast across
    partitions with rank-1 matmuls.
"""
import sys
sys.path.insert(0, "/opt/trn_rl_repo")
import numpy as np
import ml_dtypes
import jax

try:
    jax.config.update("jax_compilation_cache_dir", "/root/.jax_bass_cache")
    jax.config.update("jax_persistent_cache_min_entry_size_bytes", 0)
    jax.config.update("jax_persistent_cache_min_compile_time_secs", 0.0)
except Exception:
    pass

import concourse.bass as bass
import concourse.tile as tile
from concourse import mybir
from concourse.bass_utils import run_bass_kernel_spmd

BF16_NP = ml_dtypes.bfloat16
F32 = mybir.dt.float32
F16 = mybir.dt.float16
I16 = mybir.dt.int16
BF16 = mybir.dt.bfloat16
LOGIT_SCALE = 4096.0    # head weights are pre-scaled by this; logits ship as
                        # int16 and are divided back out on the host
AF = mybir.ActivationFunctionType
OP = mybir.AluOpType

B, L, D, H, NL, FF, W, V = 4, 8192, 256, 4, 4, 1024, 512, 14
DH = D // H
P = 128
N_CORES = 8
RB = 8            # real blocks per core
HALO = 4          # halo blocks (= NL)
NB = RB + HALO    # 12 local blocks
T = NB * W        # 6144 local tokens
TR = RB * W       # 4096 real tokens per core
DC = D // P       # 2 feature chunks

_NC = None


def _split_syncs(nc, max_waits=1, max_updates=2):
    dummy = nc.alloc_semaphore("wsplit_dummy")
    for fn in nc.m.functions:
        for blk in fn.blocks:
            out = []
            for ins in blk.instructions:
                si = ins.sync_info
                if si is None:
                    out.append(ins)
                    continue
                waits = list(si.on_wait or [])
                updates = list(si.on_update or [])
                pre = []
                while len(waits) > max_waits:
                    chunk, waits = waits[:max_waits], waits[max_waits:]
                    pre.append(mybir.InstEventSemaphore(
                        name=f"{ins.name}-ws{len(pre)}", engine=ins.engine,
                        sync_info=mybir.SyncInfo(on_wait=chunk, on_update=[
                            mybir.SyncUpdate(sync_type="semaphore", id=dummy.num,
                                             update_mode="sem-inc", update_value=1,
                                             ant_name="wsplit_dummy")])))
                post = []
                if "DMA" not in type(ins).__name__:
                    while len(updates) > max_updates:
                        chunk = updates[-max_updates:]
                        updates = updates[:-max_updates]
                        post.append(mybir.InstEventSemaphore(
                            name=f"{ins.name}-us{len(post)}", engine=ins.engine,
                            sync_info=mybir.SyncInfo(on_wait=[], on_update=chunk)))
                ins.sync_info = mybir.SyncInfo(on_wait=waits, on_update=updates)
                out.extend(pre)
                out.append(ins)
                out.extend(post)
            blk.instructions = out
    return nc


def _bcast_ap(ap, parts):
    """DRAM AP whose leading (partition) dim is a broadcast of `parts` copies."""
    return bass.AP(tensor=ap.tensor, offset=ap.offset,
                   ap=[[0, parts]] + [list(p) for p in ap.ap[1:]])


def _build():
    global _NC
    if _NC is not None:
        return _NC
    nc = bass.Bass()

    wpack_d = nc.declare_dram_parameter("wpack", [P, NL, 6144], BF16, isOutput=False)
    bcols_d = nc.declare_dram_parameter("bcols", [P, 16 * NL], F32, isOutput=False)
    bvrow_d = nc.declare_dram_parameter("bvrow", [NL, D], F32, isOutput=False)
    emb_d = nc.declare_dram_parameter("emb", [V, D], BF16, isOutput=False)
    viota_d = nc.declare_dram_parameter("viota", [V, 1], F32, isOutput=False)
    expb_d = nc.declare_dram_parameter("expb", [P, H * 640], BF16, isOutput=False)
    hw_d = nc.declare_dram_parameter("hw", [P, DC * V], BF16, isOutput=False)
    hb_d = nc.declare_dram_parameter("hb", [V, 1], F32, isOutput=False)
    ids_d = nc.declare_dram_parameter("ids", [1, T], BF16, isOutput=False)
    fcol_d = nc.declare_dram_parameter("fcol", [1, NB], F32, isOutput=False)
    lgT_d = nc.declare_dram_parameter("logitsT", [V, TR], I16, isOutput=True)

    with tile.TileContext(nc) as tc:
        with tc.tile_pool(name="const", bufs=1) as constp, \
             tc.tile_pool(name="persist", bufs=1) as pers, \
             tc.tile_pool(name="wpool", bufs=2) as wpool, \
             tc.tile_pool(name="bvp", bufs=2) as bvp, \
             tc.tile_pool(name="gelu", bufs=10) as gpool, \
             tc.tile_pool(name="ptile", bufs=3) as ptp, \
             tc.tile_pool(name="etile", bufs=2) as etp, \
             tc.tile_pool(name="work", bufs=2) as work, \
             tc.tile_pool(name="rowsf", bufs=2) as rowsf, \
             tc.tile_pool(name="rowsb", bufs=2) as rowsb, \
             tc.tile_pool(name="ltp", bufs=2) as ltp, \
             tc.tile_pool(name="psA", bufs=2, space="PSUM") as psA, \
             tc.tile_pool(name="psO", bufs=3, space="PSUM") as psO, \
             tc.tile_pool(name="psZ", bufs=1, space="PSUM") as psZ:

            # ---- constants
            emb_sb = constp.tile([V, D], BF16)
            nc.sync.dma_start(out=emb_sb, in_=emb_d[:, :])
            viota_sb = constp.tile([V, 1], F32)
            nc.sync.dma_start(out=viota_sb, in_=viota_d[:, :])
            expb_sb = constp.tile([P, H * 640], BF16)
            nc.sync.dma_start(out=expb_sb, in_=expb_d[:, :])
            bcols_sb = constp.tile([P, 16 * NL], F32)
            nc.sync.dma_start(out=bcols_sb, in_=bcols_d[:, :])
            hw_sb = constp.tile([P, DC * V], BF16)
            nc.sync.dma_start(out=hw_sb, in_=hw_d[:, :])
            hb_sb = constp.tile([V, 1], F32)
            nc.sync.dma_start(out=hb_sb, in_=hb_d[:, :])
            fsb = constp.tile([P, NB], F32)
            nc.gpsimd.dma_start(out=fsb, in_=_bcast_ap(fcol_d[0:1, :], P))
            ones1 = constp.tile([1, P], BF16)
            nc.vector.memset(ones1, 1.0)
            oneD = constp.tile([P, 1], BF16)
            nc.vector.memset(oneD, 1.0 / D)
            eps_sb = constp.tile([1, 1], F32)
            nc.vector.memset(eps_sb, 1e-5)

            # ---- persistent activation tensors (feature-partition)
            x_t = pers.tile([P, DC, T], BF16, tag="x")
            qt_t = pers.tile([P, DC, T], BF16, tag="qt")
            kt_t = pers.tile([P, DC, T], BF16, tag="kt")
            hot_t = pers.tile([P, DC, T], BF16, tag="hot")  # h / attn-out / h2
            vp_t = pers.tile([P, NB * 4 * H * 65], BF16, tag="vp")  # [tok-tile, h, 65]
            for vi in range(NB * 4):
                sl = vp_t[:, vi * 260:(vi + 1) * 260].rearrange(
                    "p (h c) -> p h c", c=65)
                nc.vector.memset(sl[:, :, 64:65], 1.0)

            # ---- embedding: one-hot matmul per 512-token slice
            for ts in range(NB):
                idsb = work.tile([V, W], BF16, tag="ids")
                nc.gpsimd.dma_start(
                    out=idsb, in_=_bcast_ap(ids_d[0:1, ts * W:(ts + 1) * W], V))
                oh = work.tile([V, W], BF16, tag="oh")
                nc.vector.tensor_scalar(oh, idsb, viota_sb, None, OP.is_equal)
                for dc in range(DC):
                    ps = psA.tile([P, W], F32, tag="big")
                    nc.tensor.matmul(ps, emb_sb[:, dc * P:(dc + 1) * P], oh,
                                     start=True, stop=True)
                    nc.vector.tensor_copy(x_t[:, dc, ts * W:(ts + 1) * W], ps)

            def layernorm(ts_range, dest_t):
                for ts in ts_range:
                    c0, c1 = ts * W, (ts + 1) * W
                    stpm = psO.tile([1, W], F32, tag="ot", name="stpm")
                    for dc in range(DC):
                        nc.tensor.matmul(stpm, oneD, x_t[:, dc, c0:c1],
                                         start=(dc == 0), stop=(dc == DC - 1))
                    stps = psO.tile([1, W], F32, tag="ot", name="stps")
                    for dc in range(DC):
                        sq = work.tile([P, W], BF16, tag="sq")
                        nc.vector.tensor_mul(sq, x_t[:, dc, c0:c1], x_t[:, dc, c0:c1])
                        nc.tensor.matmul(stps, oneD, sq,
                                         start=(dc == 0), stop=(dc == DC - 1))
                    mrow = rowsf.tile([1, W], F32, tag="mrow")
                    nc.vector.tensor_copy(mrow, stpm)
                    msq = rowsf.tile([1, W], F32, tag="msq")
                    nc.vector.tensor_mul(msq, mrow, mrow)
                    varr = rowsf.tile([1, W], F32, tag="var")
                    nc.vector.scalar_tensor_tensor(
                        varr, stps, 0.0, msq, OP.bypass, OP.subtract)
                    stdr = rowsf.tile([1, W], F32, tag="std")
                    nc.scalar.activation(stdr, varr, AF.Sqrt, bias=eps_sb)
                    ar = rowsb.tile([1, W], BF16, tag="ar")
                    with nc.allow_low_precision(reason="bf16 LN scale"):
                        nc.vector.reciprocal(ar, stdr)
                    br = rowsb.tile([1, W], BF16, tag="br")
                    nc.vector.scalar_tensor_tensor(
                        br, mrow, -1.0, ar, OP.mult, OP.mult)
                    Ab = psA.tile([P, W], F32, tag="big")
                    nc.tensor.matmul(Ab, ones1, ar, start=True, stop=True)
                    Bb = psA.tile([P, W], F32, tag="big")
                    nc.tensor.matmul(Bb, ones1, br, start=True, stop=True)
                    for dc in range(DC):
                        t1 = work.tile([P, W], BF16, tag="t1")
                        nc.vector.scalar_tensor_tensor(
                            t1, Ab, 0.0, x_t[:, dc, c0:c1], OP.bypass, OP.mult)
                        nc.vector.scalar_tensor_tensor(
                            dest_t[:, dc, c0:c1], Bb, 0.0, t1, OP.bypass, OP.add)

            # ================= layers =================
            for l in range(NL):
                l0 = l
                wl = wpool.tile([P, 6144], BF16, tag="wl")
                nc.sync.dma_start(out=wl, in_=wpack_d[:, l, :])
                bvb = bvp.tile([P, D], BF16, tag="bvb")
                nc.gpsimd.dma_start(out=bvb, in_=_bcast_ap(bvrow_d[l:l + 1, :], P))

                # ---- LN1 -> hot
                layernorm(range(l0, NB), hot_t)

                # ---- Q (blocks l0+1..), K (blocks l0..) projections
                for p3, dst, trange in ((0, qt_t, range(l0 + 1, NB)),
                                        (1, kt_t, range(l0, NB))):
                    for oc in range(DC):
                        for ts in trange:
                            ps = psA.tile([P, W], F32, tag="big")
                            for dc in range(DC):
                                tw = p3 * 4 + dc * 2 + oc
                                nc.tensor.matmul(
                                    ps, wl[:, tw * P:(tw + 1) * P],
                                    hot_t[:, dc, ts * W:(ts + 1) * W],
                                    start=(dc == 0), stop=(dc == DC - 1))
                            nc.vector.tensor_scalar(
                                dst[:, oc, ts * W:(ts + 1) * W], ps,
                                bcols_sb[:, l * 16 + p3 * 2 + oc:l * 16 + p3 * 2 + oc + 1],
                                None, OP.add)

                # ---- V projection (token-partition, per 128-token tile)
                for vi in range(4 * l0, 4 * NB):
                    ps = psA.tile([P, D], F32, tag="big")
                    for dc in range(DC):
                        nc.tensor.matmul(
                            ps, hot_t[:, dc, vi * P:(vi + 1) * P],
                            wl[:, 1536 + dc * D:1536 + (dc + 1) * D],
                            start=(dc == 0), stop=(dc == DC - 1))
                    dst = vp_t[:, vi * 260:(vi + 1) * 260].rearrange(
                        "p (h c) -> p h c", c=65)[:, :, 0:64]
                    nc.vector.scalar_tensor_tensor(
                        dst, ps.rearrange("p (h c) -> p h c", c=64), 0.0,
                        bvb.rearrange("p (h c) -> p h c", c=64),
                        OP.bypass, OP.add)

                # ---- attention, merged per key-block
                for h_ in range(H):
                    ch = h_ // 2
                    r0 = (h_ % 2) * 64
                    ot = {}
                    for n in range(l0, NB):
                        for kc in range(4):
                            kb = n * W + kc * P
                            b0 = kc * P          # piece A at tile cols [b0, W)
                            wB = kc * P + P      # piece B at tile cols [W, W+wB)
                            a0 = b0 if n > l0 else W
                            a1 = (W + wB) if n < NB - 1 else W
                            st = psA.tile([P, 1024], F32, tag="big")
                            lhsT = kt_t[r0:r0 + 64, ch, kb:kb + P]
                            if n > l0:
                                nc.tensor.matmul(
                                    st[:, b0:W], lhsT,
                                    qt_t[r0:r0 + 64, ch, kb:n * W + W],
                                    start=True, stop=True)
                            if n < NB - 1:
                                nc.tensor.matmul(
                                    st[:, W:W + wB], lhsT,
                                    qt_t[r0:r0 + 64, ch,
                                         (n + 1) * W:(n + 1) * W + wB],
                                    start=True, stop=True)
                            ep = etp.tile([P, 1024], BF16, tag="ep")
                            nc.scalar.activation(ep[:, a0:a1], st[:, a0:a1], AF.Exp)
                            pt = ptp.tile([P, 1024], BF16, tag="pt")
                            nc.vector.tensor_mul(
                                pt[:, a0:a1], ep[:, a0:a1],
                                expb_sb[:, h_ * 640 + a0 - b0:h_ * 640 + a1 - b0])
                            if n == 3 and n < NB - 1:
                                nc.vector.tensor_scalar_mul(
                                    pt[:, W:W + wB], pt[:, W:W + wB], fsb[:, 3:4])
                            vT = vp_t[:, (4 * n + kc) * 260 + h_ * 65:
                                      (4 * n + kc) * 260 + (h_ + 1) * 65]
                            if n > l0:
                                first = n not in ot
                                if first:
                                    ot[n] = psO.tile([65, W], F32, tag="ot", name=f"ot{n}")
                                nc.tensor.matmul(
                                    ot[n][:, b0:W], vT, pt[:, b0:W],
                                    start=first, stop=(kc == 3),
                                    skip_group_check=True)
                            if n < NB - 1:
                                first = (n + 1) not in ot
                                if first:
                                    ot[n + 1] = psO.tile([65, W], F32, tag="ot", name=f"ot{n+1}")
                                nc.tensor.matmul(
                                    ot[n + 1][:, 0:wB], vT, pt[:, W:W + wB],
                                    start=first, stop=False,
                                    skip_group_check=True)
                        # normalize q-block n once its last contribution landed
                        if n > l0:
                            rz = rowsb.tile([1, W], BF16, tag="rz")
                            with nc.allow_low_precision(reason="bf16 softmax z"):
                                nc.vector.reciprocal(rz, ot[n][64:65, :])
                            zb = psZ.tile([64, W], F32, tag="zb")
                            nc.tensor.matmul(zb, ones1[0:1, 0:64], rz,
                                             start=True, stop=True)
                            zbs = rowsb.tile([64, W], BF16, tag="zbs")
                            nc.vector.tensor_copy(zbs, zb)
                            nc.vector.tensor_mul(
                                hot_t[r0:r0 + 64, ch, n * W:(n + 1) * W],
                                ot[n][0:64, :], zbs)
                            del ot[n]

                # ---- O projection + residual
                for oc in range(DC):
                    for ts in range(l0 + 1, NB):
                        ps = psA.tile([P, W], F32, tag="big")
                        for dc in range(DC):
                            tw = 8 + dc * 2 + oc
                            nc.tensor.matmul(
                                ps, wl[:, tw * P:(tw + 1) * P],
                                hot_t[:, dc, ts * W:(ts + 1) * W],
                                start=(dc == 0), stop=(dc == DC - 1))
                        nc.vector.scalar_tensor_tensor(
                            x_t[:, oc, ts * W:(ts + 1) * W], ps,
                            bcols_sb[:, l * 16 + 4 + oc:l * 16 + 4 + oc + 1],
                            x_t[:, oc, ts * W:(ts + 1) * W], OP.add, OP.add)

                # ---- LN2 -> hot
                layernorm(range(l0 + 1, NB), hot_t)

                # ---- FFN
                for ts in range(l0 + 1, NB):
                    gts = []
                    for oc in range(8):
                        ps = psA.tile([P, W], F32, tag="big")
                        for dc in range(DC):
                            tw = dc * 8 + oc
                            nc.tensor.matmul(
                                ps, wl[:, 2048 + tw * P:2048 + (tw + 1) * P],
                                hot_t[:, dc, ts * W:(ts + 1) * W],
                                start=(dc == 0), stop=(dc == DC - 1))
                        gt = gpool.tile([P, W], BF16, tag="gt")
                        nc.scalar.activation(
                            gt, ps, AF.Gelu_apprx_tanh,
                            bias=bcols_sb[:, l * 16 + 8 + oc:l * 16 + 8 + oc + 1])
                        gts.append(gt)
                    for oc2 in range(DC):
                        ps2 = psO.tile([P, W], F32, tag="ot")
                        for dc in range(8):
                            tw = dc * 2 + oc2
                            nc.tensor.matmul(
                                ps2, wl[:, 4096 + tw * P:4096 + (tw + 1) * P],
                                gts[dc], start=(dc == 0), stop=(dc == 7))
                        nc.vector.scalar_tensor_tensor(
                            x_t[:, oc2, ts * W:(ts + 1) * W], ps2,
                            bcols_sb[:, l * 16 + 6 + oc2:l * 16 + 6 + oc2 + 1],
                            x_t[:, oc2, ts * W:(ts + 1) * W], OP.add, OP.add)

            # ---- final LN + head
            layernorm(range(HALO, NB), hot_t)
            for ts in range(HALO, NB):
                ps = psA.tile([V, W], F32, tag="big")
                for dc in range(DC):
                    nc.tensor.matmul(ps, hw_sb[:, dc * V:(dc + 1) * V],
                                     hot_t[:, dc, ts * W:(ts + 1) * W],
                                     start=(dc == 0), stop=(dc == DC - 1))
                lt = ltp.tile([V, W], I16, tag="lt")
                with nc.allow_low_precision(reason="int16 logits output"):
                    nc.vector.tensor_scalar(lt, ps, hb_sb, None, OP.add)
                nc.sync.dma_start(
                    out=lgT_d[:, (ts - HALO) * W:(ts - HALO + 1) * W], in_=lt)

    _split_syncs(nc)
    _NC = nc
    return nc


def nA2(kc):
    return kc * P + P


def _prep_shared(inputs):
    """Pack the (input-independent) weight tensors. Returns dict of np arrays."""
    f32 = np.float32
    emb = inputs["embed"].astype(f32)
    wq, bq = inputs["wq"].astype(f32), inputs["bq"].astype(f32)
    wk, bk = inputs["wk"].astype(f32), inputs["bk"].astype(f32)
    wv, bv = inputs["wv"].astype(f32), inputs["bv"].astype(f32)
    wo, bo = inputs["wo"].astype(f32), inputs["bo"].astype(f32)
    g1, c1 = inputs["ln1_g"].astype(f32), inputs["ln1_b"].astype(f32)
    g2, c2 = inputs["ln2_g"].astype(f32), inputs["ln2_b"].astype(f32)
    w1, b1 = inputs["w1"].astype(f32), inputs["b1"].astype(f32)
    w2, b2 = inputs["w2"].astype(f32), inputs["b2"].astype(f32)
    gf, cf = inputs["lnf_g"].astype(f32), inputs["lnf_b"].astype(f32)
    hw = inputs["head_w"].astype(f32)

    scale = f32(1.0 / np.sqrt(DH))
    wpack = np.zeros((P, NL, 6144), BF16_NP)
    bcols = np.zeros((P, 16 * NL), f32)
    bvrow = np.zeros((NL, D), f32)
    for l in range(NL):
        wqe = (g1[l][:, None] * wq[l]) * scale
        bqe = (c1[l] @ wq[l] + bq[l]) * scale
        wke = g1[l][:, None] * wk[l]
        bke = c1[l] @ wk[l] + bk[l]
        wve = g1[l][:, None] * wv[l]
        bve = c1[l] @ wv[l] + bv[l]
        w1e = g2[l][:, None] * w1[l]
        b1e = c2[l] @ w1[l] + b1[l]
        for p3, Wm in ((0, wqe), (1, wke), (2, wo[l])):
            for dc in range(DC):
                for oc in range(DC):
                    t = p3 * 4 + dc * 2 + oc
                    wpack[:, l, t * P:(t + 1) * P] = \
                        Wm[dc * P:(dc + 1) * P, oc * P:(oc + 1) * P]
        for dc in range(DC):
            wpack[:, l, 1536 + dc * D:1536 + (dc + 1) * D] = \
                wve[dc * P:(dc + 1) * P, :]
        for dc in range(DC):
            for oc in range(8):
                t = dc * 8 + oc
                wpack[:, l, 2048 + t * P:2048 + (t + 1) * P] = \
                    w1e[dc * P:(dc + 1) * P, oc * P:(oc + 1) * P]
        for dc in range(8):
            for oc in range(DC):
                t = dc * 2 + oc
                wpack[:, l, 4096 + t * P:4096 + (t + 1) * P] = \
                    w2[l][dc * P:(dc + 1) * P, oc * P:(oc + 1) * P]
        for oc in range(DC):
            bcols[:, l * 16 + 0 + oc] = bqe[oc * P:(oc + 1) * P]
            bcols[:, l * 16 + 2 + oc] = bke[oc * P:(oc + 1) * P]
            bcols[:, l * 16 + 4 + oc] = bo[l][oc * P:(oc + 1) * P]
            bcols[:, l * 16 + 6 + oc] = b2[l][oc * P:(oc + 1) * P]
        for oc in range(8):
            bcols[:, l * 16 + 8 + oc] = b1e[oc * P:(oc + 1) * P]
        bvrow[l] = bve

    hwe = (gf[:, None] * hw) * LOGIT_SCALE
    hbe = (cf @ hw) * LOGIT_SCALE
    hwp = np.zeros((P, DC * V), BF16_NP)
    for dc in range(DC):
        hwp[:, dc * V:(dc + 1) * V] = hwe[dc * P:(dc + 1) * P, :]

    slopes = np.exp2(-8.0 * np.arange(1, H + 1, dtype=f32) / H)
    pp = np.arange(P, dtype=f32)[:, None]
    tt = np.arange(640, dtype=f32)[None, :]
    dist = tt - pp
    expb = np.zeros((P, H * 640), BF16_NP)
    for h_ in range(H):
        eb = np.exp(-slopes[h_] * dist) * ((dist >= 0) & (dist <= W))
        expb[:, h_ * 640:(h_ + 1) * 640] = eb.astype(BF16_NP)

    shared = {
        "wpack": wpack,
        "bcols": bcols,
        "bvrow": bvrow,
        "emb": emb.astype(BF16_NP),
        "viota": np.arange(V, dtype=f32)[:, None],
        "expb": expb,
        "hw": hwp,
        "hb": hbe[:, None].astype(f32),
    }
    return shared


def _fcol_concat():
    fcol = np.ones((N_CORES, NB), np.float32)
    for c in range(N_CORES):
        for n in range(NB):
            g = RB * c + (n + 1) - HALO
            if g % (L // W) == 0:
                fcol[c, n] = 0.0
    return fcol


def _ids_concat(byte_ids):
    """Per-core halo'd token streams, concatenated on axis 0: [N_CORES, T]."""
    flat_ids = np.asarray(byte_ids).reshape(-1)
    NT = flat_ids.shape[0]
    out = np.empty((N_CORES, T), BF16_NP)
    for c in range(N_CORES):
        idx = (np.arange(c * TR - HALO * W, c * TR + TR)) % NT
        out[c] = flat_ids[idx].astype(BF16_NP)
    return out


def _prep(inputs):
    """Fallback-path per-core input maps (original contract)."""
    shared = _prep_shared(inputs)
    ids = _ids_concat(inputs["byte_ids"])
    fcol = _fcol_concat()
    in_maps = []
    for c in range(N_CORES):
        m = dict(shared)
        m["ids"] = ids[c:c + 1]
        m["fcol"] = fcol[c:c + 1]
        in_maps.append(m)
    return in_maps


# ---------------------------------------------------------------------------
# Fast dispatch: compile the shard_map executable once, keep the (static)
# packed weights resident on the 8 devices, and per invocation only upload the
# ids stream (197KB), execute, and fetch the logits. The previous invocation's
# output buffer is donated back as the kernel's output-init operand (the
# kernel writes every element of logitsT, so its initial contents are dead).
# ---------------------------------------------------------------------------
_DISP = None


def _make_dispatch(shared):
    import jax.numpy as jnp
    from jax.sharding import Mesh, PartitionSpec, NamedSharding
    from jax.experimental.shard_map import shard_map
    from concourse.bass2jax import (
        _bass_exec_p, partition_id_tensor, install_neuronx_cc_hook)

    nc = _build()
    install_neuronx_cc_hook()

    part_name = nc.partition_id_tensor.name if nc.partition_id_tensor else None
    in_names, out_names, out_avals = [], [], []
    for alloc in nc.m.functions[0].allocations:
        if not isinstance(alloc, mybir.MemoryLocationSet):
            continue
        name = alloc.memorylocations[0].name
        if alloc.kind == "ExternalInput":
            if name != part_name:
                in_names.append(name)
        elif alloc.kind == "ExternalOutput":
            out_names.append(name)
            out_avals.append(jax.core.ShapedArray(
                tuple(alloc.tensor_shape), mybir.dt.np(alloc.dtype)))
    n_params, n_outs = len(in_names), len(out_avals)
    all_in = in_names + out_names + ([part_name] if part_name else [])

    def _body(*args):
        operands = list(args)
        if part_name is not None:
            operands.append(partition_id_tensor())
        return tuple(_bass_exec_p.bind(
            *operands, out_avals=tuple(out_avals), in_names=tuple(all_in),
            out_names=tuple(out_names), lowering_input_output_aliases=(),
            sim_require_finite=True, sim_require_nnan=True, nc=nc))

    devices = jax.devices()[:N_CORES]
    mesh = Mesh(np.asarray(devices), ("core",))
    shard = NamedSharding(mesh, PartitionSpec("core"))
    donate = tuple(range(n_params, n_params + n_outs))
    sharded = jax.jit(
        shard_map(_body, mesh=mesh,
                  in_specs=(PartitionSpec("core"),) * (n_params + n_outs),
                  out_specs=(PartitionSpec("core"),) * n_outs,
                  check_rep=False),
        donate_argnums=donate, keep_unused=True)

    ids_idx = in_names.index("ids")
    fcol = _fcol_concat()
    dev_in = []
    for nm in in_names:
        if nm == "ids":
            arr = np.zeros((N_CORES, T), BF16_NP)
        elif nm == "fcol":
            arr = fcol
        else:
            a = shared[nm]
            arr = np.concatenate([a] * N_CORES, axis=0)
        dev_in.append(jax.device_put(arr, shard))
    (ov,) = out_avals
    out0 = jax.jit(lambda: jnp.zeros((N_CORES * ov.shape[0],) + ov.shape[1:],
                                     ov.dtype), out_shardings=shard)()
    jax.block_until_ready(dev_in)
    return {"sharded": sharded, "dev_in": dev_in, "ids_idx": ids_idx,
            "shard": shard, "out": out0, "ov": ov, "shared": shared}


def _ensure_dispatch(shared):
    global _DISP
    if _DISP is not None:
        prev = _DISP["shared"]
        if prev is shared or \
                all(np.array_equal(prev[k], shared[k]) for k in shared):
            return _DISP
        _DISP = None                    # weights changed: rebuild residency
    _DISP = _make_dispatch(shared)
    return _DISP


def _invoke(ids_np):
    """One full device invocation: upload ids, execute on 8 cores, fetch and
    assemble the [B, L, V] logits on host."""
    d = _DISP
    ids_dev = jax.device_put(ids_np, d["shard"])
    args = list(d["dev_in"])
    args[d["ids_idx"]] = ids_dev
    out = d["sharded"](*args, d["out"])[0]
    d["out"] = out
    arr = np.asarray(out)                              # [8*V, TR] int16
    arr = arr.reshape(N_CORES, V, TR).transpose(0, 2, 1).astype(np.float32)
    arr *= 1.0 / LOGIT_SCALE
    return arr.reshape(B, L, V)


_PREP_CACHE = {}


def _weights_digest(inputs):
    import hashlib
    h = hashlib.blake2b(digest_size=16)
    for k in sorted(inputs):
        if k != "byte_ids":
            a = np.ascontiguousarray(inputs[k])
            h.update(k.encode());  h.update(a.tobytes())
    return h.hexdigest()


def kernel(**inputs):
    inputs = {k: np.asarray(v) for k, v in inputs.items()}
    try:
        dig = _weights_digest(inputs)
        if dig in _PREP_CACHE:
            shared = _PREP_CACHE[dig]
        else:
            shared = _prep_shared(inputs)
            _PREP_CACHE.clear()
            _PREP_CACHE[dig] = shared
        _ensure_dispatch(shared)
        ids = _ids_concat(inputs["byte_ids"])
        return _invoke(ids).astype(np.float32)
    except Exception:
        import traceback
        traceback.print_exc()
        # conservative fallback: original run_bass_kernel_spmd path
        in_maps = _prep(inputs)
        nc = _build()
        res = run_bass_kernel_spmd(nc, in_maps, list(range(N_CORES)))
        parts = [res.results[c]["logitsT"].T for c in range(N_CORES)]
        out = np.concatenate(parts, axis=0).reshape(B, L, V).astype(np.float32)
        return out * (1.0 / LOGIT_SCALE)

